# revision 5
# baseline (speedup 1.0000x reference)
"""Causal attention with clipped softmax on 8 TRN2 NeuronCores.

Problem: S=4096, H=16, D=128, B=1, fp32 inputs.
  scores = Q K^T / sqrt(D), causal mask, softmax,
  probs = clip(1.03*softmax - 0.03, 0, 1)   (== relu since upper clip never binds)
  out = probs @ V

Sharding: 2 heads per core (tensor parallel over heads), no collectives.

Per-core kernel (per head, per 128-row q-tile i, kv = 128*(i+1)):
  1. QK^T in fp32r (full PE rate at N>=256): psum scores [q=128, chunk<=1024]
  2. ACT Exp psum->SBUF bf16, accum_out gives running row sums Z (free)
  3. diagonal block masked multiplicatively (DVE tensor_tensor_reduce,
     which also fixes that chunk's Z contribution)
  4. clip folded: t = relu(e - (0.03/1.03) Z) on DVE (tensor_scalar sub+max),
     final out rows scaled by 1.03/Z (per-partition, DVE)
  5. PE transpose t blocks (bf16, via identity) -> psum, DVE copyback
  6. PV: out[q,d] += tT_kb.T @ V_kb accumulated in psum (bf16 matmuls)
"""

import math

import numpy as np
import ml_dtypes

S = 4096
H = 16
D = 128
N_CORES = 8
HPC = H // N_CORES  # heads per core
NQT = S // 128  # 32 q-tiles per head
SCALE = 1.0 / math.sqrt(D)
GAMMA = -0.03
ZETA = 1.0
A = ZETA - GAMMA  # 1.03
CHUNK = 1024  # scores chunk width (psum tile: 2 banks)
TGROUP = 4  # transpose blocks batched per psum tile / copyback

_CACHE = {}


def _build():
    import concourse.bass as bass  # noqa: F401
    import concourse.mybir as mybir
    import concourse.tile as tile
    from concourse import bacc
    from concourse.masks import make_identity

    dt = mybir.dt
    f32 = dt.float32
    bf16 = dt.bfloat16

    nc = bacc.Bacc("TRN2", target_bir_lowering=False, debug=False, num_devices=N_CORES)

    qt_d = nc.dram_tensor("qt", [HPC, 128, S], bf16, kind="ExternalInput")
    kt_d = nc.dram_tensor("kt", [HPC, 128, S], bf16, kind="ExternalInput")
    v_d = nc.dram_tensor("v", [HPC, 128, NQT, 128], bf16, kind="ExternalInput")
    o_d = nc.dram_tensor("o", [HPC, S, D], f32, kind="ExternalOutput")

    with tile.TileContext(nc) as tc:
        with (
            tc.tile_pool(name="const", bufs=1) as constp,
            tc.tile_pool(name="qk", bufs=2) as qkpool,
            tc.tile_pool(name="vp", bufs=2) as vpool,
            tc.tile_pool(name="ep", bufs=2) as epool,
            tc.tile_pool(name="tp", bufs=2) as tpool,
            tc.tile_pool(name="ttp", bufs=4) as ttpool,
            tc.tile_pool(name="zp", bufs=3) as zpool,
            tc.tile_pool(name="op", bufs=3) as opool,
            tc.tile_pool(name="ps_s", bufs=2, space="PSUM") as ps_s,
            tc.tile_pool(name="ps_t", bufs=2, space="PSUM") as ps_t,
            tc.tile_pool(name="ps_o", bufs=2, space="PSUM") as ps_o,
        ):
            ident = constp.tile([128, 128], bf16)
            make_identity(nc, ident[:])
            # multiplicative causal mask for the diagonal 128x128 block:
            # cmask[x, y] = 1.0 if x >= y else 0.0
            cmask = constp.tile([128, 128], bf16)
            nc.gpsimd.memset(cmask[:], 1.0)
            nc.gpsimd.affine_select(
                out=cmask[:],
                in_=cmask[:],
                compare_op=mybir.AluOpType.is_ge,
                fill=0.0,
                base=0,
                pattern=[[-1, 128]],
                channel_multiplier=1,
            )

            for h in range(HPC):
                qt_sb = qkpool.tile([128, S], bf16, tag="qt")
                kt_sb = qkpool.tile([128, S], bf16, tag="kt")
                v_sb = vpool.tile([128, NQT, 128], bf16, tag="v")
                nc.sync.dma_start(qt_sb[:], qt_d.ap()[h])
                kchunk = min(1024, S)
                for kc in range(S // kchunk):
                    nc.sync.dma_start(
                        kt_sb[:, kc * kchunk : (kc + 1) * kchunk],
                        kt_d.ap()[h, :, kc * kchunk : (kc + 1) * kchunk],
                    )
                nc.sync.dma_start(v_sb[:], v_d.ap()[h])

                state = {}

                def stage_a(i):
                    kv = 128 * (i + 1)
                    e = epool.tile([128, S], bf16, tag="e")
                    zp = zpool.tile([128, 8], f32, tag="zpart")
                    qslice = qt_sb[:, i * 128 : (i + 1) * 128]
                    ncol = 0  # accum columns used
                    c0 = 0
                    while c0 < kv:
                        cn = min(CHUNK, kv - c0)
                        ps = ps_s.tile([128, CHUNK], f32, tag="s")
                        # QK^T chunk: matmuls of <=512 cols into one psum tile
                        m0 = 0
                        while m0 < cn:
                            mn = min(512, cn - m0)
                            nc.tensor.matmul(
                                ps[:, m0 : m0 + mn],
                                qslice,
                                kt_sb[:, c0 + m0 : c0 + m0 + mn],
                                start=True,
                                stop=True,
                            )
                            m0 += mn
                        # exp (keep the diagonal 128 in a separate call: its
                        # accum would include to-be-masked elements)
                        main = cn - 128 if c0 + cn == kv else cn
                        if main > 0:
                            nc.scalar.activation(
                                e[:, c0 : c0 + main],
                                ps[:, :main],
                                mybir.ActivationFunctionType.Exp,
                                scale=SCALE,
                                accum_out=zp[:, ncol : ncol + 1],
                            )
                            ncol += 1
                        if main < cn:
                            nc.scalar.activation(
                                e[:, kv - 128 : kv],
                                ps[:, main:cn],
                                mybir.ActivationFunctionType.Exp,
                                scale=SCALE,
                            )
                        c0 += cn
                    # mask diagonal block, then repair its Z contribution
                    nc.vector.tensor_mul(
                        e[:, kv - 128 : kv], e[:, kv - 128 : kv], cmask[:]
                    )
                    nc.vector.tensor_reduce(
                        zp[:, ncol : ncol + 1],
                        e[:, kv - 128 : kv],
                        axis=mybir.AxisListType.X,
                        op=mybir.AluOpType.add,
                    )
                    ncol += 1
                    state[i] = (e, zp, ncol)

                def stage_b(i):
                    kv = 128 * (i + 1)
                    e, zp, ncol = state.pop(i)
                    zsum = zpool.tile([128, 1], f32, tag="zsum")
                    nc.vector.tensor_reduce(
                        zsum[:], zp[:, :ncol], axis=mybir.AxisListType.X,
                        op=mybir.AluOpType.add,
                    )
                    cbias = zpool.tile([128, 1], f32, tag="cbias")
                    nc.vector.tensor_scalar_mul(cbias[:], zsum[:], GAMMA / -A)
                    zinv = zpool.tile([128, 1], f32, tag="zinv")
                    nc.vector.reciprocal(zinv[:], zsum[:])
                    ascale = zpool.tile([128, 1], f32, tag="ascale")
                    nc.vector.tensor_scalar_mul(ascale[:], zinv[:], A)
                    # t = relu(e - cbias)
                    t = tpool.tile([128, S], bf16, tag="t")
                    nc.vector.tensor_scalar(
                        out=t[:, :kv],
                        in0=e[:, :kv],
                        scalar1=cbias[:],
                        scalar2=0.0,
                        op0=mybir.AluOpType.subtract,
                        op1=mybir.AluOpType.max,
                    )
                    # transpose + PV
                    ops = ps_o.tile([128, 128], f32, tag="o")
                    nkb = i + 1
                    kb = 0
                    while kb < nkb:
                        g = min(TGROUP, nkb - kb)
                        tps = ps_t.tile([128, TGROUP * 128], bf16, tag="tt")
                        for j in range(g):
                            nc.tensor.transpose(
                                tps[:, j * 128 : (j + 1) * 128],
                                t[:, (kb + j) * 128 : (kb + j + 1) * 128],
                                ident[:],
                            )
                        tts = ttpool.tile([128, TGROUP * 128], bf16, tag="tts")
                        nc.vector.tensor_copy(tts[:, : g * 128], tps[:, : g * 128])
                        for j in range(g):
                            nc.tensor.matmul(
                                ops[:],
                                tts[:, j * 128 : (j + 1) * 128],
                                v_sb[:, kb + j, :],
                                start=(kb + j == 0),
                                stop=(kb + j == nkb - 1),
                                skip_group_check=True,
                            )
                        kb += g
                    osb = opool.tile([128, D], f32, tag="osb")
                    nc.vector.tensor_scalar_mul(osb[:], ops[:], ascale[:])
                    nc.sync.dma_start(o_d.ap()[h, i * 128 : (i + 1) * 128, :], osb[:])

                # software pipeline: keep PE busy during softmax of tile i
                stage_a(0)
                for i in range(1, NQT):
                    stage_a(i)
                    stage_b(i - 1)
                stage_b(NQT - 1)

    nc.compile()
    return nc


def _get_nc():
    if "nc" not in _CACHE:
        _CACHE["nc"] = _build()
    return _CACHE["nc"]


def kernel(query_states, key_states, value_states, q_sequence_mask, kv_sequence_mask):
    from concourse import bass_utils

    nc = _get_nc()

    q = np.asarray(query_states, dtype=np.float32)
    k = np.asarray(key_states, dtype=np.float32)
    v = np.asarray(value_states, dtype=np.float32)

    in_maps = []
    for c in range(N_CORES):
        hs = slice(HPC * c, HPC * (c + 1))
        # [S, hpc, D] -> [hpc, D, S]
        qt = np.ascontiguousarray(q[:, hs, :].transpose(1, 2, 0)).astype(
            ml_dtypes.bfloat16
        )
        kt = np.ascontiguousarray(k[:, hs, :].transpose(1, 2, 0)).astype(
            ml_dtypes.bfloat16
        )
        # [S, hpc, D] -> [hpc, S, D] -> [hpc, kb, p, D] -> [hpc, p, kb, D]
        vc = (
            v[:, hs, :]
            .transpose(1, 0, 2)
            .reshape(HPC, NQT, 128, D)
            .transpose(0, 2, 1, 3)
        )
        vc = np.ascontiguousarray(vc).astype(ml_dtypes.bfloat16)
        in_maps.append({"qt": qt, "kt": kt, "v": vc})

    res = bass_utils.run_bass_kernel_spmd(
        nc, in_maps, core_ids=list(range(N_CORES))
    )

    out = np.empty((S, H, D), dtype=np.float32)
    for c in range(N_CORES):
        oc = res.results[c]["o"]  # [hpc, S, D]
        for hh in range(HPC):
            out[:, HPC * c + hh, :] = oc[hh]
    return out
